# revision 11
# baseline (speedup 1.0000x reference)
"""Trainium2 Bass kernel for DisentangledSelfAttention (8-core data parallel).

Math (from the reference):
  Q = query @ Wq ; K = key @ Wk ; V = value @ Wv + bv     (per-head split)
  Qc = Q - mean_fields(Q)                                 (bq cancels)
  pairwise = softmax(Qc K^T)  per (batch, head)           (K-centering and bk
      drop: softmax is over the key axis == the axis K's mean is taken over,
      so the shift is softmax-invariant)
  unary softmax over a size-1 axis == 1, so
  out = relu(pairwise @ V + colsum(V) + query)

Sharding: batch (2048) split across 8 cores, 256 batches/core, weights
replicated. Each core streams its 16384-row block in 32 blocks of 512 rows
(8 batches).

Precision/scale scheme: q/k/v stream in as fp8-e4m3 (transposed layout),
weights as 64*W in fp8 (keeps W out of the e4m3 subnormal range). All three
projections run as fp8 DoubleRow matmuls (two 128-row k-tiles per
instruction). The x64 scale rides through QK^T (lg = 4096*logits, descaled
by the exp's scale=1/4096) and through PV (V tiles are 64*V with the
softmax-denominator ones-column set to 64.0, so the normalize divide cancels
the scale). colsum(V) is computed exactly from a host-side sum_f(value)
fp16 tensor so fp8 noise in V cannot touch the dominant output term;
Q-centering uses a host-side -sum_f(query) fp16 tensor projected on-chip
with fp16 weights (tiny matmuls), no on-chip reductions anywhere.

Every stationary operand spans partition row 0 (sub-row-offset stationaries
fault on this toolchain); per-head attention stationaries use zero-padded
block-diagonal layouts as before.
"""

import sys
from contextlib import ExitStack

sys.path.insert(0, "/opt/trn_rl_repo")

import numpy as np
import ml_dtypes

import concourse.bacc as bacc
import concourse.tile as tile
from concourse import mybir

B, F, D = 2048, 64, 512
A, H, HD = 512, 8, 64
NCORES = 8
BL = B // NCORES          # batches per core
M = BL * F                # rows per core
MB = 512                  # rows per block (8 batches)
NB_FULL = M // MB         # 32 blocks

F32 = mybir.dt.float32
F16 = mybir.dt.float16
F8 = mybir.dt.float8e4
AF = mybir.ActivationFunctionType
DR = mybir.MatmulPerfMode.DoubleRow
NPF8 = ml_dtypes.float8_e4m3fn


def bcast_inner(ap2d, inner):
    """[P, n] -> [P, n, inner] with stride-0 inner axis."""
    return ap2d.rearrange("p (b x) -> p b x", x=1).broadcast_to(
        [ap2d.shape[0], ap2d.shape[1], inner]
    )


def build_program(nblocks=NB_FULL, stage=6):
    nc = bacc.Bacc("TRN2", target_bir_lowering=False, debug=False,
                   num_devices=NCORES)
    m_tot = nblocks * MB

    q8 = nc.dram_tensor("q8", [128, nblocks * 2048], F8, kind="ExternalInput").ap()
    k8 = nc.dram_tensor("k8", [128, nblocks * 2048], F8, kind="ExternalInput").ap()
    v8 = nc.dram_tensor("v8", [128, nblocks * 2048], F8, kind="ExternalInput").ap()
    qn = nc.dram_tensor("qn", [128, nblocks * 2048], F16, kind="ExternalInput").ap()
    sxqn = nc.dram_tensor("sxqn", [128, nblocks * 32], F16,
                          kind="ExternalInput").ap()
    sxv = nc.dram_tensor("sxv", [128, nblocks * 32], F16,
                         kind="ExternalInput").ap()
    wq8 = nc.dram_tensor("wq8", [128, 2048], F8, kind="ExternalInput").ap()
    wk8 = nc.dram_tensor("wk8", [128, 2048], F8, kind="ExternalInput").ap()
    wv8m = nc.dram_tensor("wv8m", [128, 2048], F8, kind="ExternalInput").ap()
    wq16 = nc.dram_tensor("wq16", [128, 2048], F16, kind="ExternalInput").ap()
    wv16 = nc.dram_tensor("wv16", [128, 2048], F16, kind="ExternalInput").ap()
    bv64 = nc.dram_tensor("bv64", [1, A], F16, kind="ExternalInput").ap()
    id128 = nc.dram_tensor("id128", [128, 128], F16, kind="ExternalInput").ap()
    bsel = nc.dram_tensor("bsel", [8, A], F16, kind="ExternalInput").ap()
    ones1 = nc.dram_tensor("ones1", [1, 128], F16, kind="ExternalInput").ap()
    ones8 = nc.dram_tensor("ones8", [1, 8], F16, kind="ExternalInput").ap()
    out = nc.dram_tensor("out", [128, nblocks * 2048], F16,
                         kind="ExternalOutput").ap()

    with tile.TileContext(nc) as tc, ExitStack() as ctx:
        const = ctx.enter_context(tc.tile_pool(name="const", bufs=1))
        p_in = ctx.enter_context(tc.tile_pool(name="p_in", bufs=3))
        p_stat = ctx.enter_context(tc.tile_pool(name="p_stat", bufs=2))
        p_q = ctx.enter_context(tc.tile_pool(name="p_q", bufs=2))
        p_fin = ctx.enter_context(tc.tile_pool(name="p_fin", bufs=2))
        p_out = ctx.enter_context(tc.tile_pool(name="p_out", bufs=2))
        ps_proj = ctx.enter_context(tc.tile_pool(name="ps_proj", bufs=3,
                                                 space="PSUM"))
        ps_lg = ctx.enter_context(tc.tile_pool(name="ps_lg", bufs=1,
                                               space="PSUM"))
        ps_o = ctx.enter_context(tc.tile_pool(name="ps_o", bufs=2, space="PSUM"))
        ps_sm = ctx.enter_context(tc.tile_pool(name="ps_sm", bufs=1,
                                               space="PSUM"))

        # --- constants ---
        w_sb = {}
        for name, ap in (("q8", wq8), ("k8", wk8), ("v8m", wv8m)):
            t = const.tile([128, 2048], F8, tag=f"w{name}")
            nc.sync.dma_start(t[:], ap[:])
            w_sb[name] = t
        for name, ap in (("q16", wq16), ("v16", wv16)):
            t = const.tile([128, 2048], F16, tag=f"w{name}")
            nc.sync.dma_start(t[:], ap[:])
            w_sb[name] = t
        sxqn_sb = const.tile([128, nblocks * 32], F16, tag="sxqn")
        nc.sync.dma_start(sxqn_sb[:], sxqn[:])
        sxv_sb = const.tile([128, nblocks * 32], F16, tag="sxv")
        nc.sync.dma_start(sxv_sb[:], sxv[:])
        bv64_sb = const.tile([1, A], F16, tag="bv64")
        nc.sync.dma_start(bv64_sb[:], bv64[:])
        id128_sb = const.tile([128, 128], F16, tag="id128")
        nc.sync.dma_start(id128_sb[:], id128[:])
        bsel_sb = const.tile([8, A], F16, tag="bsel")
        nc.sync.dma_start(bsel_sb[:], bsel[:])
        ones1_sb = const.tile([1, 128], F16, tag="ones1")
        nc.sync.dma_start(ones1_sb[:], ones1[:])
        ones8_sb = const.tile([1, 8], F16, tag="ones8")
        nc.sync.dma_start(ones8_sb[:], ones8[:])
        neg8_sb = const.tile([128, 1], F32, tag="neg8")
        nc.vector.memset(neg8_sb[:], -8.0)

        # block-diagonal rings: zero (or 64.0) regions are set once and never
        # overwritten by the per-block writes.
        kc_ring = []
        for r in range(2):
            t = const.tile([128, 4096], F16, tag=f"kc{r}")
            nc.gpsimd.memset(
                t[64:128, :].rearrange("p (x c) -> p x c", c=256)[:, :, 0:128],
                0.0)
            nc.gpsimd.memset(
                t[0:64, :].rearrange("p (x c) -> p x c", c=256)[:, :, 128:256],
                0.0)
            kc_ring.append(t)
        pt_ring = []
        for r in range(3):
            t = const.tile([128, 1024], F16, tag=f"ptr{r}")
            nc.gpsimd.memset(
                t[0:64, :].rearrange("p (h c) -> p h c", c=128)[:, :, 64:128],
                0.0)
            nc.gpsimd.memset(
                t[64:128, :].rearrange("p (h c) -> p h c", c=128)[:, :, 0:64],
                0.0)
            pt_ring.append(t)
        v16_ring = []
        for r in range(2):
            t = const.tile([128, 4 * H * 65], F16, tag=f"v16r{r}")
            nc.gpsimd.memset(
                t[:].rearrange("p (x c) -> p x c", c=65)[:, :, 64:65], 64.0)
            v16_ring.append(t)

        def emit_dmas(bi):
            x8 = {}
            for name, src in (("q", q8), ("k", k8), ("v", v8)):
                t = p_in.tile([128, 2048], F8, tag=f"x8{name}")
                nc.sync.dma_start(t[:], src[:, bi * 2048:(bi + 1) * 2048])
                x8[name] = t
            qn_t = p_in.tile([128, 2048], F16, tag="qn")
            nc.sync.dma_start(qn_t[:], qn[:, bi * 2048:(bi + 1) * 2048])
            return dict(bi=bi, x8=x8, qn_t=qn_t)

        def proj_units(st):
            """14 closures: nmu, cs, 4x Q, 4x K, 4x V."""
            bi, x8 = st["bi"], st["x8"]

            def nmu_unit():
                # nmuq[a, b] = -(sum_f xq)^T Wq = -64*muQ^T, per A-chunk fc
                ps = ps_sm.tile([128, 512], F32, tag="sm")
                for fc in range(4):
                    for c in range(4):
                        nc.tensor.matmul(
                            ps[:, fc * 8:(fc + 1) * 8],
                            w_sb["q16"][:, c * 512 + fc * 128:
                                        c * 512 + fc * 128 + 128],
                            sxqn_sb[:, bi * 32 + c * 8:bi * 32 + (c + 1) * 8],
                            start=(c == 0), stop=(c == 3))
                nmuq = p_stat.tile([128, 32], F16, tag="nmuq")
                nc.vector.tensor_copy(nmuq[:], ps[:, 0:32])
                st["nmuq"] = nmuq

            def cs_unit():
                # cs16[b, :] = sum_f V[b, f, :] = sxv[b] @ Wv + 64*bv  (exact)
                ps = ps_sm.tile([128, 512], F32, tag="sm")
                nc.tensor.matmul(ps[0:8, :], ones8_sb[:], bv64_sb[:],
                                 start=True, stop=False)
                for c in range(4):
                    nc.tensor.matmul(
                        ps[0:8, :],
                        sxv_sb[:, bi * 32 + c * 8:bi * 32 + (c + 1) * 8],
                        w_sb["v16"][:, c * 512:(c + 1) * 512],
                        start=False, stop=(c == 3))
                cs16 = p_stat.tile([8, A], F16, tag="cs16")
                nc.scalar.copy(cs16[:], ps[0:8, :])
                st["cs16"] = cs16

            def q_unit(fc):
                def emit():
                    ps = ps_proj.tile([128, MB], F32, tag="psP")
                    for g in range(2):
                        nc.tensor.matmul(
                            ps[:],
                            w_sb["q8"][:, (g * 4 + fc) * 256:
                                       (g * 4 + fc) * 256 + 256].rearrange(
                                "p (i a) -> p i a", i=2),
                            x8["q"][:, g * 1024:(g + 1) * 1024].rearrange(
                                "p (i m) -> p i m", i=2),
                            start=(g == 0), stop=(g == 1), perf_mode=DR)
                    # centered fp16 copy: qc16 = ps + (-64 muQ) broadcast
                    nc.vector.tensor_add(
                        st["qc16"][:, fc * 512:(fc + 1) * 512].rearrange(
                            "p (b f) -> p b f", f=F),
                        ps[:].rearrange("p (b f) -> p b f", f=F),
                        bcast_inner(st["nmuq"][:, fc * 8:(fc + 1) * 8], F))
                return emit

            def k_unit(fc):
                def emit():
                    ps = ps_proj.tile([128, MB], F32, tag="psP")
                    for g in range(2):
                        nc.tensor.matmul(
                            ps[:],
                            w_sb["k8"][:, (g * 4 + fc) * 256:
                                       (g * 4 + fc) * 256 + 256].rearrange(
                                "p (i a) -> p i a", i=2),
                            x8["k"][:, g * 1024:(g + 1) * 1024].rearrange(
                                "p (i m) -> p i m", i=2),
                            start=(g == 0), stop=(g == 1), perf_mode=DR)
                    # parity-padded fp16 copies (no centering needed):
                    # per (fc, j) two 128-col stationaries, head-even data on
                    # rows 0:64 (hr=0), head-odd on rows 64:128 (hr=1).
                    kc16 = st["kc16"]
                    sect = kc16[:, fc * 1024:(fc + 1) * 1024]
                    nc.scalar.copy(
                        sect[0:64, :].rearrange(
                            "p (x c) -> p x c", c=256)[:, :, 0:128],
                        ps[0:64, :].rearrange("p (x c) -> p x c", c=128))
                    nc.scalar.copy(
                        sect[64:128, :].rearrange(
                            "p (x c) -> p x c", c=256)[:, :, 128:256],
                        ps[64:128, :].rearrange("p (x c) -> p x c", c=128))
                return emit

            def v_unit(mt):
                def emit():
                    ps = ps_proj.tile([128, A], F32, tag="psP")
                    nc.tensor.matmul(ps[:], ones1_sb[:], bv64_sb[:],
                                     start=True, stop=False)
                    for g in range(2):
                        nc.tensor.matmul(
                            ps[:],
                            x8["v"][:, g * 1024:(g + 1) * 1024].rearrange(
                                "p (i m) -> p i m", i=2)[:, :,
                                                         mt * 128:(mt + 1) * 128],
                            w_sb["v8m"][:, g * 1024:(g + 1) * 1024].rearrange(
                                "p (i a) -> p i a", i=2),
                            start=False, stop=(g == 1), perf_mode=DR)
                    v16 = st["v16"]
                    nc.vector.tensor_copy(
                        v16[:, mt * 520:(mt + 1) * 520].rearrange(
                            "p (h c) -> p h c", c=65)[:, :, 0:64],
                        ps[:].rearrange("p (h c) -> p h c", c=64))
                return emit

            st["qc16"] = p_q.tile([128, 2048], F16, tag="qc16", name="qc16")
            st["kc16"] = kc_ring[bi % 2]
            st["v16"] = v16_ring[bi % 2]
            units = [nmu_unit, cs_unit]
            for fc in range(4):
                units.append(q_unit(fc))
                units.append(k_unit(fc))
            for mt in range(4):
                units.append(v_unit(mt))
            return units

        def emit_back(st, fill_units):
            """Attention + finalize for a block whose projections are done.
            fill_units (next block's projection closures) are interleaved so
            the PE stream always has ready matmul work during softmax."""
            bi = st["bi"]
            qc16, kc16, v16, qn_t = (st["qc16"], st["kc16"], st["v16"],
                                     st["qn_t"])
            lg_t = {}
            fill = list(fill_units)

            def do_fill(n):
                for _ in range(n):
                    if fill:
                        fill.pop(0)()

            def do_qk(j):
                # one [128,128] parity-padded stationary per head covers both
                # batches of the j-pair; off-diagonal quadrants of the [128,
                # 128] output are cross-batch garbage that exp never reads.
                lg = ps_lg.tile([128, 1024], F32, tag="lg")
                for h in range(H):
                    hp, hr = h // 2, h % 2
                    nc.tensor.matmul(
                        lg[:, h * 128:(h + 1) * 128],
                        kc16[:, hp * 1024 + j * 256 + hr * 128:
                             hp * 1024 + j * 256 + (hr + 1) * 128],
                        qc16[:, hp * 512 + j * 128:hp * 512 + (j + 1) * 128],
                        start=True, stop=True)
                lg_t[j] = lg

            do_qk(0)
            for j in range(4):
                lg = lg_t.pop(j)
                # exp((lg/4096) - 8) -> fp16 block-diagonal per batch parity.
                # 1/4096 descalews the x64 q and k tiles; -8 keeps exp in
                # fp16 range (softmax shift-invariant; logits reach ~12).
                pt_z = pt_ring[(bi * 4 + j) % 3]
                hi = pt_z[0:64, :].rearrange("p (h c) -> p h c", c=128)
                lo = pt_z[64:128, :].rearrange("p (h c) -> p h c", c=128)
                nc.scalar.activation(
                    hi[:, :, 0:64],
                    lg[0:64, :].rearrange(
                        "p (h c) -> p h c", c=128)[:, :, 0:64], AF.Exp,
                    bias=neg8_sb[0:64, :], scale=1.0 / 4096.0)
                nc.scalar.activation(
                    lo[:, :, 64:128],
                    lg[64:128, :].rearrange(
                        "p (h c) -> p h c", c=128)[:, :, 64:128], AF.Exp,
                    bias=neg8_sb[64:128, :], scale=1.0 / 4096.0)
                do_fill(3)

                oA = ps_o.tile([128, 512], F32, tag="o")
                oB = ps_o.tile([128, 512], F32, tag="o")
                for h in range(H):
                    o = oA if h < 4 else oB
                    oc = (h % 4) * 65
                    nc.tensor.matmul(
                        o[:, oc:oc + 65],
                        pt_z[:, h * 128:(h + 1) * 128],
                        v16[:, j * 520 + h * 65:j * 520 + (h + 1) * 65],
                        start=True, stop=True)
                if j + 1 < 4:
                    do_qk(j + 1)
                rz = p_stat.tile([128, 8], F32, tag="rz")
                nc.vector.reciprocal(
                    rz[:, 0:4],
                    oA[:, 0:260].rearrange("p (h c) -> p h c", c=65)[:, :, 64])
                nc.vector.reciprocal(
                    rz[:, 4:8],
                    oB[:, 0:260].rearrange("p (h c) -> p h c", c=65)[:, :, 64])
                fin = p_fin.tile([128, A], F32, tag=f"fin{j}")
                nc.vector.tensor_mul(
                    fin[:, 0:256].rearrange("p (h q) -> p h q", q=64),
                    oA[:, 0:260].rearrange("p (h c) -> p h c", c=65)[:, :, 0:64],
                    bcast_inner(rz[:, 0:4], 64))
                nc.vector.tensor_mul(
                    fin[:, 256:512].rearrange("p (h q) -> p h q", q=64),
                    oB[:, 0:260].rearrange("p (h c) -> p h c", c=65)[:, :, 0:64],
                    bcast_inner(rz[:, 4:8], 64))

                # finalize j: + colsum(V) + query, relu, into the out tile
                qv = ps_proj.tile([128, A], F32, tag="psP")
                nc.tensor.matmul(qv[:], bsel_sb[:, j * 128:(j + 1) * 128],
                                 st["cs16"][:], start=True, stop=False)
                nc.tensor.matmul(qv[:], id128_sb[:],
                                 qn_t[:, j * 512:(j + 1) * 512],
                                 start=False, stop=True)
                nc.vector.tensor_add(fin[:], fin[:], qv[:])
                if j % 2 == 0:
                    nc.scalar.activation(
                        st["o16"][:, j * 512:(j + 1) * 512], fin[:], AF.Relu)
                else:
                    nc.vector.tensor_scalar_max(
                        st["o16"][:, j * 512:(j + 1) * 512], fin[:], 0.0)
                do_fill(1)
            nc.sync.dma_start(out[:, bi * 2048:(bi + 1) * 2048], st["o16"][:])
            do_fill(99)



        st0 = emit_dmas(0)
        st0["o16"] = p_out.tile([128, 2048], F16, tag="o16", name="o16")
        for u in proj_units(st0):
            u()
        prev = st0
        for bi in range(1, nblocks):
            cur = emit_dmas(bi)
            cur["o16"] = p_out.tile([128, 2048], F16, tag="o16", name="o16")
            emit_back(prev, proj_units(cur))
            prev = cur
        emit_back(prev, [])

    nc.compile()
    return nc


def _to_blocked_T(x, nblocks):
    """[m, D] -> transposed blocked layout [128, nblocks*4*MB]."""
    m = nblocks * MB
    return np.ascontiguousarray(
        x[:m].T.reshape(4, 128, nblocks, MB).transpose(1, 2, 0, 3).reshape(
            128, nblocks * 2048))


def _to_blocked_rows(x, nblocks):
    """[m, D] -> natural-rows blocked layout [128, nblocks*4*D]."""
    m = nblocks * MB
    return np.ascontiguousarray(
        x[:m].reshape(nblocks, 4, 128, D).transpose(2, 0, 1, 3).reshape(
            128, nblocks * 2048))


def _sx_layout(sx, nblocks):
    """[bl, D] -> [128, nblocks*4*8] (bi, c, b) layout."""
    bl = nblocks * MB // F
    return np.ascontiguousarray(
        sx[:bl].T.reshape(4, 128, nblocks, 8).transpose(1, 2, 0, 3).reshape(
            128, nblocks * 32))


def make_in_map(query, key, value, Wq, Wk, Wv, bv, core, nblocks=NB_FULL):
    """Build one core's input dict. query/key/value are the FULL arrays."""
    sl = slice(core * BL, (core + 1) * BL)
    xq = np.asarray(query[sl], np.float32).reshape(BL * F, D)
    xk = np.asarray(key[sl], np.float32).reshape(BL * F, D)
    xv = np.asarray(value[sl], np.float32).reshape(BL * F, D)
    Wq = np.asarray(Wq, np.float32)
    Wk = np.asarray(Wk, np.float32)
    Wv = np.asarray(Wv, np.float32)
    bv = np.asarray(bv, np.float32)

    sxq = xq.reshape(BL, F, D).sum(1)
    sxv_h = xv.reshape(BL, F, D).sum(1)

    w8 = lambda w: np.ascontiguousarray(
        (64.0 * w).reshape(2, 2, 128, 4, 128).transpose(2, 0, 3, 1, 4).reshape(
            128, 2048)).astype(NPF8)
    w16 = lambda w: np.ascontiguousarray(
        w.reshape(4, 128, A).transpose(1, 0, 2).reshape(128, 2048)).astype(
            np.float16)

    bsel = np.zeros((8, A), np.float16)
    for j in range(4):
        bsel[2 * j, j * 128:j * 128 + 64] = 1.0
        bsel[2 * j + 1, j * 128 + 64:(j + 1) * 128] = 1.0

    return {
        "q8": _to_blocked_T(xq, nblocks).astype(NPF8),
        "k8": _to_blocked_T(xk, nblocks).astype(NPF8),
        "v8": _to_blocked_T(xv, nblocks).astype(NPF8),
        "qn": _to_blocked_rows(xq, nblocks).astype(np.float16),
        "sxqn": _sx_layout(-sxq, nblocks).astype(np.float16),
        "sxv": _sx_layout(sxv_h, nblocks).astype(np.float16),
        "wq8": w8(Wq),
        "wk8": w8(Wk),
        "wv8m": np.ascontiguousarray(
            (64.0 * Wv).reshape(2, 2, 128, A).transpose(2, 0, 1, 3).reshape(
                128, 2048)).astype(NPF8),
        "wq16": w16(Wq),
        "wv16": w16(Wv),
        "bv64": (64.0 * bv).reshape(1, A).astype(np.float16),
        "id128": np.eye(128, dtype=np.float16),
        "bsel": bsel,
        "ones1": np.ones((1, 128), np.float16),
        "ones8": np.ones((1, 8), np.float16),
    }


def out_to_rows(o, nblocks=NB_FULL):
    """[128, nblocks*2048] fp16 -> [m, A] fp32."""
    return o.reshape(128, nblocks, 4, A).transpose(1, 2, 0, 3).reshape(
        nblocks * MB, A).astype(np.float32)


_CACHED_NC = None


def kernel(query, key, value, Wq, bq, Wk, bk, Wv, bv, Wk2, bk2):
    """Full-input kernel: shards batch over 8 NeuronCores, returns full output.

    bq cancels under field-mean centering; bk and K-centering drop because the
    pairwise softmax is over the key axis (shift-invariant); Wk2/bk2 drop
    because the unary softmax is over a size-1 axis. All are accepted unused.
    """
    global _CACHED_NC
    from concourse.bass_utils import run_bass_kernel_spmd

    if _CACHED_NC is None:
        _CACHED_NC = build_program()
    in_maps = [make_in_map(query, key, value, Wq, Wk, Wv, bv, c)
               for c in range(NCORES)]
    res = run_bass_kernel_spmd(_CACHED_NC, in_maps,
                               core_ids=list(range(NCORES)), trace=False)
    parts = [out_to_rows(res.results[c]["out"]).reshape(BL, F, A)
             for c in range(NCORES)]
    return np.concatenate(parts, axis=0)
